# revision 48
# baseline (speedup 1.0000x reference)
"""Multi-head causal attention with RoPE on 8 Trainium2 NeuronCores.

Sharding: core c = 2*b + g handles batch b (of 4) and head-group g (of 2,
8 heads each).  Each core computes its 8 heads' attention and a partial
output projection (against its column-slice of wo); the host sums the two
partials per batch.

Per-core kernel layout notes:
 - All matmul inputs live in SBUF as fp32 and are bitcast to float32r for
   the PE (full-rate at N>=256, near-fp32 storage).
 - q/k head dims are permuted host-side (folded into wq/wk rows) so the
   RoPE rotate-half becomes a 16-row block swap that stream_shuffle can do
   in one DVE pass.  Scores are invariant to any fixed dim permutation.
 - Scores are computed transposed (keys on partitions, queries free) as
   two concurrent K=64 row-tiled matmuls (PE tile rows 0:64 / 64:128, one
   per head of the pair), so softmax's denominator comes free from an
   extra ones-column appended to V in the attn@V matmul, and exp() fuses
   with PSUM eviction on ScalarE.
 - Causal masking: key-tiles fully behind the query block are skipped, the
   triangular corner accumulates -2^30 via one ident@maskadd matmul.
 - Softmax division is deferred: the un-normalized attn@V output (queries
   on the free dim) is scaled by 1/denom broadcast across partitions via a
   DRAM-bounce DMA, fused into the PSUM eviction multiply.
 - The output projection for query-block J is emitted two head-pair blocks
   into the NEXT J-group's attention, so the PE queue never stalls on the
   softmax-denominator chains.
"""

import sys

sys.path.insert(0, "/opt/trn_rl_repo")

import numpy as np

D_MODEL = 1024
NUM_HEADS = 16
D_K = 64
B_FULL, S = 4, 2048
THETA = 10000.0
N_CORES = 8
H_CORE = 8  # heads per core
HP = 4      # head pairs per core
SB = 4      # 512-wide s-blocks
ST = 16     # 128-wide s-tiles
KT = 8      # 128-deep k-tiles over D_MODEL

# stream_shuffle applies its 32-entry mask within each 32-partition block:
# this swaps the two 16-row halves of every block.
SHUF16 = list(range(16, 32)) + list(range(0, 16))

_CACHE = {}


def _build_module(mm="float32r", taps=False):
    import concourse.bacc as bacc
    import concourse.tile as tile
    from concourse import mybir
    from contextlib import ExitStack

    P = 128
    FP32 = mybir.dt.float32
    MMD = getattr(mybir.dt, mm)
    EXP = mybir.ActivationFunctionType.Exp

    nc = bacc.Bacc("TRN2", target_bir_lowering=False, debug=False,
                   num_devices=N_CORES)

    xT = nc.dram_tensor("xT", [D_MODEL, S], MMD, kind="ExternalInput")
    wqT = nc.dram_tensor("wqT", [D_MODEL, 512], MMD, kind="ExternalInput")
    wkT = nc.dram_tensor("wkT", [D_MODEL, 512], MMD, kind="ExternalInput")
    wvT = nc.dram_tensor("wvT", [D_MODEL, 512], MMD, kind="ExternalInput")
    woT = nc.dram_tensor("woT", [512, D_MODEL], MMD, kind="ExternalInput")
    cosT = nc.dram_tensor("cosT", [P, S], FP32, kind="ExternalInput")
    sinT = nc.dram_tensor("sinT", [P, S], FP32, kind="ExternalInput")
    BF16 = mybir.dt.bfloat16
    maskA = nc.dram_tensor("maskA", [P, P], BF16, kind="ExternalInput")
    identT = nc.dram_tensor("identT", [P, P], BF16, kind="ExternalInput")
    outD = nc.dram_tensor("out", [S, D_MODEL], FP32, kind="ExternalOutput")
    if taps:
        tq = nc.dram_tensor("tap_qt0", [P, S], MMD, kind="ExternalOutput")
        tk = nc.dram_tensor("tap_kt0", [P, S], MMD, kind="ExternalOutput")
        tv = nc.dram_tensor("tap_v", [P, ST, H_CORE, 65], MMD,
                            kind="ExternalOutput")
        to = nc.dram_tensor("tap_oT0", [P, S], MMD, kind="ExternalOutput")
    # denominator bounce buffers: raw rows in, reciprocal rows out;
    # chains run per hp-pair (4 rows at a time)
    scrD = nc.dram_tensor("scrD", [SB, 2, 4, 512], FP32)
    scrR = nc.dram_tensor("scrR", [SB, 2, 4, 512], FP32)

    xT3 = xT.rearrange("(ko p) s -> p ko s", p=P)
    wqT3 = wqT.rearrange("(ko p) m -> p ko m", p=P)
    wkT3 = wkT.rearrange("(ko p) m -> p ko m", p=P)
    wvT3 = wvT.rearrange("(ko p) m -> p ko m", p=P)
    woT3 = woT.rearrange("(t p) n -> p t n", p=P)

    with tile.TileContext(nc) as tc:
        with ExitStack() as ctx:
            const_pool = ctx.enter_context(tc.tile_pool(name="const", bufs=1))
            qk_pool = ctx.enter_context(tc.tile_pool(name="qk", bufs=1))
            v_pool = ctx.enter_context(tc.tile_pool(name="vp", bufs=1))

            maskadd_sb = const_pool.tile([P, P], BF16, name="maskadd_sb")
            ident_sb = const_pool.tile([P, P], BF16, name="ident_sb")
            nc.gpsimd.dma_start(out=maskadd_sb[:], in_=maskA[:, :])
            nc.gpsimd.dma_start(out=ident_sb[:], in_=identT[:, :])

            qt = [qk_pool.tile([P, S], MMD, tag=f"qt{i}", name=f"qt{i}")
                  for i in range(HP)]
            # per-pair kt [128, S]: rows 0:64 head h2=0, 64:128 h2=1;
            # scores contract K=64 via concurrent PE row tiles.
            kt = [qk_pool.tile([P, S], MMD, tag=f"kt{i}", name=f"kt{i}")
                  for i in range(HP)]
            v_sb = v_pool.tile([P, ST, H_CORE, 65], MMD)

            # ---------------- Phase A: QKV projections + RoPE ----------------
            # three passes (Q, K, V) so only one weight + x-stream are
            # resident at a time; x is re-streamed per pass
            with ExitStack() as actx:
                wpool = actx.enter_context(tc.tile_pool(name="wts", bufs=2))
                xpool = actx.enter_context(tc.tile_pool(name="xs", bufs=2))
                cspool = actx.enter_context(tc.tile_pool(name="cs", bufs=1))
                rpool = actx.enter_context(tc.tile_pool(name="rope", bufs=2))
                psA = actx.enter_context(
                    tc.tile_pool(name="psA", bufs=6, space="PSUM"))

                cos_sb = cspool.tile([P, S], FP32, tag="cos", name="cos_sb")
                sin_sb = cspool.tile([P, S], FP32, tag="sin", name="sin_sb")
                nc.gpsimd.dma_start(out=cos_sb[:], in_=cosT[:, :])
                nc.gpsimd.dma_start(out=sin_sb[:], in_=sinT[:, :])
                # ones column (index 64) of every per-head V' block
                ones_c = cspool.tile([P, ST, H_CORE, 1], FP32, tag="ones",
                                     name="ones_c")
                nc.vector.memset(ones_c[:], 1.0)
                nc.scalar.copy(v_sb[:, :, :, 64:65], ones_c[:])

                for wdram, mode in ((wqT3, "q"), (wkT3, "k"),
                                    (wvT3, "v")):
                    w_sb = wpool.tile([P, KT, 512], MMD, tag="w",
                                      name=f"w_{mode}")
                    for k in range(KT):
                        nc.sync.dma_start(out=w_sb[:, k, :],
                                            in_=wdram[:, k, :])
                    for sb in range(SB):
                        sbs = slice(sb * 512, (sb + 1) * 512)
                        xs = xpool.tile([P, KT, 512], MMD, tag="xs",
                                        name="xs")
                        # split so the k-loop can start on the first slice
                        for kq in range(4):
                            nc.sync.dma_start(
                                out=xs[:, 2 * kq:2 * kq + 2, :],
                                in_=xT3[:, 2 * kq:2 * kq + 2, sbs])
                        if mode in ("q", "k"):
                            for hp in range(HP):
                                hps = slice(hp * 128, (hp + 1) * 128)
                                ps = psA.tile([P, 512], FP32, tag="pa",
                                              name="pa")
                                for k in range(KT):
                                    nc.tensor.matmul(
                                        ps[:, :],
                                        w_sb[:, k, hps],
                                        xs[:, k, :],
                                        start=(k == 0), stop=(k == KT - 1),
                                    )
                                rot = rpool.tile([P, 512], FP32, tag="rot",
                                                 name="rot")
                                nc.vector.stream_shuffle(rot[:], ps[:, :],
                                                         mask=SHUF16)
                                t1 = rpool.tile([P, 512], FP32, tag="t1",
                                                name="t1")
                                nc.vector.tensor_mul(t1[:], ps[:, :],
                                                     cos_sb[:, sbs])
                                t2 = rpool.tile([P, 512], FP32, tag="t2",
                                                name="t2")
                                nc.vector.tensor_mul(t2[:], rot[:],
                                                     sin_sb[:, sbs])
                                if mode == "q":
                                    nc.vector.tensor_add(qt[hp][:, sbs],
                                                         t1[:], t2[:])
                                else:
                                    nc.vector.tensor_add(kt[hp][:, sbs],
                                                         t1[:], t2[:])
                        else:
                            for sti in range(4):
                                st = sb * 4 + sti
                                stp = slice(sti * 128, (sti + 1) * 128)
                                psv = psA.tile([P, 512], FP32, tag="pa",
                                               name="psv")
                                for k in range(KT):
                                    nc.tensor.matmul(
                                        psv[:, :],
                                        xs[:, k, stp],
                                        w_sb[:, k, :],
                                        start=(k == 0), stop=(k == KT - 1),
                                    )
                                pv = psv[:, :].rearrange(
                                    "p (h d) -> p h d", h=8)
                                # every head: [V | 1]
                                nc.scalar.copy(v_sb[:, st, :, 0:64],
                                               pv[:, :, :])

            # ---------------- Phase B: attention ----------------
            ot_pool = ctx.enter_context(tc.tile_pool(name="otp", bufs=1))
            oT = [ot_pool.tile([P, S], MMD, tag=f"oT{i}", name=f"oT{i}")
                  for i in range(HP)]
            wopool = ctx.enter_context(tc.tile_pool(name="wo", bufs=1))
            wo_sb = wopool.tile([P, 4, D_MODEL], MMD)
            nc.sync.dma_start(out=wo_sb[:], in_=woT3[:, :, :])

            with ExitStack() as bctx:
                epool = bctx.enter_context(tc.tile_pool(name="expp", bufs=4))
                rdpool = bctx.enter_context(tc.tile_pool(name="rdp", bufs=2))
                dspool = bctx.enter_context(tc.tile_pool(name="dsp", bufs=1))
                bcpool = bctx.enter_context(tc.tile_pool(name="bcp", bufs=2))
                psS = bctx.enter_context(
                    tc.tile_pool(name="psS", bufs=2, space="PSUM"))
                # po tiles are staged out to SBUF right after attn@V, so a
                # single buffer per (hp-parity, head) suffices
                psO = bctx.enter_context(
                    tc.tile_pool(name="psO", bufs=1, space="PSUM"))

                def nlo_of(I, J):
                    r = I - 4 * J
                    return 128 * r if r >= 0 else 0

                opool = bctx.enter_context(
                    tc.tile_pool(name="ostage", bufs=1))

                def emit_outproj_st(st):
                    # output projection for one 128-query s-tile; psum
                    # borrowed from the scores pool (same tag)
                    stp = slice(st * 128, (st + 1) * 128)
                    pc = psS.tile([P, 2, 512], FP32, tag="psS", name="pc")
                    for nb in range(2):
                        nbs = slice(nb * 512, (nb + 1) * 512)
                        for t in range(4):
                            nc.tensor.matmul(
                                pc[:, nb, :],
                                oT[t][:, stp],
                                wo_sb[:, t, nbs],
                                start=(t == 0), stop=(t == 3),
                            )
                    ob = opool.tile([P, 2, 512], FP32, tag="ob", name="ob")
                    nc.vector.tensor_copy(ob[:], pc[:, :])
                    nc.gpsimd.dma_start(
                        out=outD[stp, :],
                        in_=ob[:, :, :].rearrange("p a b -> p (a b)"))

                # round-robin over head pairs; the output projection for a
                # J-group is deferred into the NEXT group (one s-tile per
                # hp-block) so the PE never waits on the softmax-denominator
                # chains.  Denominator reciprocals are batched per J-group:
                # the 8 [1,512] rows are staged, DMA-gathered onto 8
                # partitions, and inverted with ONE DVE reciprocal (its cost
                # is free-size-bound, so 8 rows cost the same as 1).
                jseq = (0, 1, 2, 3)
                pending = []

                def pop_outproj():
                    if pending:
                        emit_outproj_st(pending.pop(0))

                for jidx, J in enumerate(jseq):
                    Js = slice(J * 512, (J + 1) * 512)
                    # [V|1] attn@V results staged out of PSUM per (hp,h2):
                    # rows 0:64 head dims, row 64 the softmax denominator
                    stage = dspool.tile([P, 8, 512], FP32, tag="dstage",
                                        name="dstage")

                    def emit_chain(half):
                        # softmax-denominator chain for head pairs
                        # (2*half, 2*half+1): gather the 4 staged rows onto
                        # partitions 0:4 via a DRAM bounce, invert once
                        # (DVE reciprocal cost is free-size-bound), bounce
                        # back out for the per-row partition broadcasts.
                        hs = slice(4 * half, 4 * half + 4)
                        nc.sync.dma_start(out=scrD[J, half, :, :],
                                          in_=stage[64:65, hs, :])
                        dg = rdpool.tile([4, 512], FP32, tag="dg",
                                         name="dg")
                        nc.sync.dma_start(out=dg[0:4, :],
                                          in_=scrD[J, half, :, :])
                        rd = rdpool.tile([4, 512], FP32, tag="rd",
                                         name="rd")
                        nc.vector.reciprocal(rd[0:4, :], dg[0:4, :])
                        nc.sync.dma_start(out=scrR[J, half, :, :],
                                          in_=rd[0:4, :])
                        for hp in (2 * half, 2 * half + 1):
                            for h2 in range(2):
                                idx = hp * 2 + h2
                                bc = bcpool.tile([P, 512], FP32, tag="bc",
                                                 name="bc")
                                eng = nc.gpsimd if h2 == 0 else nc.sync
                                eng.dma_start(
                                    out=bc[0:64, :],
                                    in_=scrR[J, half, idx - 4 * half, :]
                                    .partition_broadcast(64))
                                if h2 == 0:
                                    nc.vector.tensor_mul(
                                        oT[hp][0:64, Js],
                                        stage[0:64, idx, :],
                                        bc[0:64, :])
                                else:
                                    # normalized evict lands at partitions
                                    # 0-63; DMA shifts it into oT's upper
                                    # half
                                    tmp = rdpool.tile([P, 512], MMD,
                                                      tag="tmpb",
                                                      name="tmpb")
                                    nc.vector.tensor_mul(
                                        tmp[0:64, :],
                                        stage[0:64, idx, :],
                                        bc[0:64, :])
                                    nc.sync.dma_start(
                                        out=oT[hp][64:128, Js],
                                        in_=tmp[0:64, :])

                    # two head-pairs' I-loops interleave so the PE fills
                    # one stream's exp latency with the other stream's
                    # matmuls (the inner loop is otherwise ACT-bound)
                    n_i = 4 * J + 4
                    for pair in range(2):
                        hps = (2 * pair, 2 * pair + 1)
                        po = {hp: [psO.tile([P, 512], FP32,
                                            tag=f"po{hp % 2}{h2}",
                                            name=f"po{hp % 2}{h2}")
                                   for h2 in range(2)] for hp in hps}
                        def emit_attnv(I, exs):
                            nlo = nlo_of(I, J)
                            for hp in hps:
                                for h2 in range(2):
                                    # attn @ [V|1]: rows 0..63 dims,
                                    # row 64 denom
                                    nc.tensor.matmul(
                                        po[hp][h2][0:65, nlo:],
                                        v_sb[:, I, hp * 2 + h2, :],
                                        exs[hp][:, h2, nlo:],
                                        start=(I == 0), stop=(I == n_i - 1),
                                    )

                        # attn@V trails the scores by one I-step so the PE
                        # fills the exp latency with the next step's scores
                        prev = None
                        for I in range(n_i):
                            if I == n_i // 2:
                                pop_outproj()
                            nlo = nlo_of(I, J)
                            ks = slice(I * 128, (I + 1) * 128)
                            qs = slice(J * 512 + nlo, (J + 1) * 512)
                            diag = I - 4 * J >= 0
                            exs = {}
                            for hp in hps:
                                ps = psS.tile([P, 2, 512], FP32, tag="psS",
                                              name="psS")
                                for h2 in range(2):
                                    hr = slice(h2 * 64, (h2 + 1) * 64)
                                    nc.tensor.matmul(
                                        ps[:, h2, nlo:],
                                        kt[hp][hr, ks],
                                        qt[hp][hr, qs],
                                        start=True, stop=not diag,
                                    )
                                    if diag:
                                        # causal corner: accumulate -2^30
                                        # into masked (q < k) entries
                                        nc.tensor.matmul(
                                            ps[:, h2, nlo:nlo + 128],
                                            ident_sb[:, :],
                                            maskadd_sb[:, :],
                                            start=False, stop=True,
                                        )
                                ex = epool.tile([P, 2, 512], MMD, tag="ex",
                                                name="ex")
                                nc.scalar.activation(ex[:, :, nlo:],
                                                     ps[:, :, nlo:],
                                                     EXP, scale=0.125)
                                exs[hp] = ex
                            if prev is not None:
                                emit_attnv(prev[0], prev[1])
                            prev = (I, exs)
                        emit_attnv(prev[0], prev[1])
                        for hp in hps:
                            for h2 in range(2):
                                # free the PSUM slot right away: dims +
                                # denom staged to SBUF in one copy
                                nc.scalar.copy(
                                    stage[0:65, hp * 2 + h2, :],
                                    po[hp][h2][0:65, :])
                        emit_chain(pair)
                        pop_outproj()
                    pending.extend(range(4 * J, 4 * J + 4))
                while pending:
                    pop_outproj()

            if taps:
                nc.gpsimd.dma_start(out=tq[:, :], in_=qt[0][:])
                nc.gpsimd.dma_start(out=tk[:, :], in_=kt[0][:])
                nc.gpsimd.dma_start(out=tv[:, :, :, :], in_=v_sb[:])
                nc.gpsimd.dma_start(out=to[:, :], in_=oT[0][:])

    nc.compile()
    return nc


def get_module(mm="float32r"):
    if mm not in _CACHE:
        _CACHE[mm] = _build_module(mm)
    return _CACHE[mm]


def _head_perm():
    """Within-head dim permutation: 16-pair blocks [x1 x2 x1 x2]."""
    p = []
    for blk in range(2):
        base = blk * 32
        p += [2 * (base // 2 + i) for i in range(16)]       # x1 of pairs
        p += [2 * (base // 2 + i) + 1 for i in range(16)]   # x2 of pairs
    return np.array(p)


def prep_core_inputs(inputs, mm="float32r"):
    import ml_dtypes
    mdt = np.float32 if mm != "bfloat16" else ml_dtypes.bfloat16
    x = np.asarray(inputs["x"], dtype=np.float32)
    tp = np.asarray(inputs["token_positions"])
    wq = np.asarray(inputs["wq"], dtype=np.float32)
    wk = np.asarray(inputs["wk"], dtype=np.float32)
    wv = np.asarray(inputs["wv"], dtype=np.float32)
    wo = np.asarray(inputs["wo"], dtype=np.float32)

    import ml_dtypes
    perm = _head_perm()
    qi = np.arange(128)[None, :]
    ki = np.arange(128)[:, None]
    mask_add = np.where(qi < ki, -np.float32(2.0 ** 30),
                        np.float32(0.0)).astype(ml_dtypes.bfloat16)
    ident = np.eye(128, dtype=ml_dtypes.bfloat16)

    freqs = 1.0 / THETA ** (np.arange(0, D_K, 2, dtype=np.float32) / D_K)

    in_maps = []
    for c in range(N_CORES):
        b, g = divmod(c, 2)
        rows = slice(g * 512, (g + 1) * 512)
        wq_g = wq[rows].reshape(H_CORE, D_K, D_MODEL)[:, perm, :]
        wk_g = wk[rows].reshape(H_CORE, D_K, D_MODEL)[:, perm, :]

        pos = tp[b].astype(np.float32)
        ang = freqs[:, None] * pos[None, :]          # [32, S]
        cos32, sin32 = np.cos(ang), np.sin(ang)
        # permuted row l: l%32 < 16 -> x1 of pair (l%32 + 16*(l//32)),
        #                 else x2 of the same pair; x1 rows get -sin.
        cos64 = np.concatenate([cos32[0:16], cos32[0:16],
                                cos32[16:32], cos32[16:32]], axis=0)
        sin64 = np.concatenate([-sin32[0:16], sin32[0:16],
                                -sin32[16:32], sin32[16:32]], axis=0)
        cosT = np.tile(cos64, (2, 1))
        sinT = np.tile(sin64, (2, 1))

        in_maps.append({
            "xT": np.ascontiguousarray(x[b].T).astype(mdt),
            "wqT": np.ascontiguousarray(wq_g.reshape(512, D_MODEL).T).astype(mdt),
            "wkT": np.ascontiguousarray(wk_g.reshape(512, D_MODEL).T).astype(mdt),
            "wvT": np.ascontiguousarray(wv[rows].T).astype(mdt),
            "woT": np.ascontiguousarray(wo[:, rows].T).astype(mdt),
            "cosT": np.ascontiguousarray(cosT),
            "sinT": np.ascontiguousarray(sinT),
            "maskA": mask_add,
            "identT": ident,
        })
    return in_maps


DEFAULT_MM = "float32r"


def kernel(**inputs):
    from concourse.bass_utils import run_bass_kernel_spmd

    import os
    mm = os.environ.get("KMM", DEFAULT_MM)
    nc = get_module(mm)
    in_maps = prep_core_inputs(inputs, mm)
    res = run_bass_kernel_spmd(nc, in_maps, core_ids=list(range(N_CORES)))
    out = np.empty((B_FULL, S, D_MODEL), np.float32)
    for b in range(B_FULL):
        out[b] = res.results[2 * b]["out"] + res.results[2 * b + 1]["out"]
    return out


# revision 51
# speedup vs baseline: 1.1888x; 1.1888x over previous
"""Multi-head causal attention with RoPE on 8 Trainium2 NeuronCores.

Sharding: core c = 2*b + g handles batch b (of 4) and head-group g (of 2,
8 heads each).  Each core computes its 8 heads' attention and a partial
output projection (against its column-slice of wo); the host sums the two
partials per batch.

Per-core kernel layout notes:
 - All matmul inputs live in SBUF as fp32 and are bitcast to float32r for
   the PE (full-rate at N>=256, near-fp32 storage).
 - q/k head dims are permuted host-side (folded into wq/wk rows) so the
   RoPE rotate-half becomes a 16-row block swap that stream_shuffle can do
   in one DVE pass.  Scores are invariant to any fixed dim permutation.
 - Scores are computed transposed (keys on partitions, queries free) as
   two concurrent K=64 row-tiled matmuls (PE tile rows 0:64 / 64:128, one
   per head of the pair), so softmax's denominator comes free from an
   extra ones-column appended to V in the attn@V matmul, and exp() fuses
   with PSUM eviction on ScalarE.
 - Causal masking: key-tiles fully behind the query block are skipped, the
   triangular corner accumulates -2^30 via one ident@maskadd matmul.
 - Softmax division is deferred: the un-normalized attn@V output (queries
   on the free dim) is scaled by 1/denom broadcast across partitions via a
   DRAM-bounce DMA, fused into the PSUM eviction multiply.
 - The output projection for query-block J is emitted two head-pair blocks
   into the NEXT J-group's attention, so the PE queue never stalls on the
   softmax-denominator chains.
"""

import sys

sys.path.insert(0, "/opt/trn_rl_repo")

import numpy as np

D_MODEL = 1024
NUM_HEADS = 16
D_K = 64
B_FULL, S = 4, 2048
THETA = 10000.0
N_CORES = 8
H_CORE = 8  # heads per core
HP = 4      # head pairs per core
SB = 4      # 512-wide s-blocks
ST = 16     # 128-wide s-tiles
KT = 8      # 128-deep k-tiles over D_MODEL

# stream_shuffle applies its 32-entry mask within each 32-partition block:
# this swaps the two 16-row halves of every block.
SHUF16 = list(range(16, 32)) + list(range(0, 16))

_CACHE = {}


def _build_module(mm="float32r", taps=False):
    import concourse.bacc as bacc
    import concourse.tile as tile
    from concourse import mybir
    from contextlib import ExitStack

    P = 128
    FP32 = mybir.dt.float32
    MMD = getattr(mybir.dt, mm)
    EXP = mybir.ActivationFunctionType.Exp

    nc = bacc.Bacc("TRN2", target_bir_lowering=False, debug=False,
                   num_devices=N_CORES)

    xT = nc.dram_tensor("xT", [D_MODEL, S], MMD, kind="ExternalInput")
    wqT = nc.dram_tensor("wqT", [D_MODEL, 512], MMD, kind="ExternalInput")
    wkT = nc.dram_tensor("wkT", [D_MODEL, 512], MMD, kind="ExternalInput")
    wvT = nc.dram_tensor("wvT", [D_MODEL, 512], MMD, kind="ExternalInput")
    woT = nc.dram_tensor("woT", [512, D_MODEL], MMD, kind="ExternalInput")
    cosT = nc.dram_tensor("cosT", [P, S], FP32, kind="ExternalInput")
    sinT = nc.dram_tensor("sinT", [P, S], FP32, kind="ExternalInput")
    BF16 = mybir.dt.bfloat16
    maskA = nc.dram_tensor("maskA", [P, P], BF16, kind="ExternalInput")
    identT = nc.dram_tensor("identT", [P, P], BF16, kind="ExternalInput")
    outD = nc.dram_tensor("out", [S, D_MODEL], FP32, kind="ExternalOutput")
    if taps:
        tq = nc.dram_tensor("tap_qt0", [P, S], MMD, kind="ExternalOutput")
        tk = nc.dram_tensor("tap_kt0", [P, S], MMD, kind="ExternalOutput")
        tv = nc.dram_tensor("tap_v", [P, ST, H_CORE, 65], MMD,
                            kind="ExternalOutput")
        to = nc.dram_tensor("tap_oT0", [P, S], MMD, kind="ExternalOutput")
    # denominator bounce buffers: raw rows in, reciprocal rows out;
    # chains run per hp-pair (4 rows at a time)
    scrD = nc.dram_tensor("scrD", [SB, 2, 4, 512], FP32)
    scrR = nc.dram_tensor("scrR", [SB, 2, 4, 512], FP32)

    xT3 = xT.rearrange("(ko p) s -> p ko s", p=P)
    wqT3 = wqT.rearrange("(ko p) m -> p ko m", p=P)
    wkT3 = wkT.rearrange("(ko p) m -> p ko m", p=P)
    wvT3 = wvT.rearrange("(ko p) m -> p ko m", p=P)
    woT3 = woT.rearrange("(t p) n -> p t n", p=P)

    with tile.TileContext(nc) as tc:
        with ExitStack() as ctx:
            const_pool = ctx.enter_context(tc.tile_pool(name="const", bufs=1))
            qk_pool = ctx.enter_context(tc.tile_pool(name="qk", bufs=1))
            v_pool = ctx.enter_context(tc.tile_pool(name="vp", bufs=1))

            maskadd_sb = const_pool.tile([P, P], BF16, name="maskadd_sb")
            ident_sb = const_pool.tile([P, P], BF16, name="ident_sb")
            nc.gpsimd.dma_start(out=maskadd_sb[:], in_=maskA[:, :])
            nc.gpsimd.dma_start(out=ident_sb[:], in_=identT[:, :])

            qt = [qk_pool.tile([P, S], MMD, tag=f"qt{i}", name=f"qt{i}")
                  for i in range(HP)]
            # per-pair kt [128, S]: rows 0:64 head h2=0, 64:128 h2=1;
            # scores contract K=64 via concurrent PE row tiles.
            kt = [qk_pool.tile([P, S], MMD, tag=f"kt{i}", name=f"kt{i}")
                  for i in range(HP)]
            v_sb = v_pool.tile([P, ST, H_CORE, 65], MMD)

            # ---------------- Phase A: QKV projections + RoPE ----------------
            # three passes (Q, K, V) so only one weight + x-stream are
            # resident at a time; x is re-streamed per pass
            with ExitStack() as actx:
                wpool = actx.enter_context(tc.tile_pool(name="wts", bufs=2))
                xpool = actx.enter_context(tc.tile_pool(name="xs", bufs=2))
                cspool = actx.enter_context(tc.tile_pool(name="cs", bufs=1))
                rpool = actx.enter_context(tc.tile_pool(name="rope", bufs=2))
                psA = actx.enter_context(
                    tc.tile_pool(name="psA", bufs=6, space="PSUM"))

                cos_sb = cspool.tile([P, S], FP32, tag="cos", name="cos_sb")
                sin_sb = cspool.tile([P, S], FP32, tag="sin", name="sin_sb")
                nc.gpsimd.dma_start(out=cos_sb[:], in_=cosT[:, :])
                nc.gpsimd.dma_start(out=sin_sb[:], in_=sinT[:, :])
                # ones column (index 64) of every per-head V' block
                ones_c = cspool.tile([P, ST, H_CORE, 1], FP32, tag="ones",
                                     name="ones_c")
                nc.vector.memset(ones_c[:], 1.0)
                nc.scalar.copy(v_sb[:, :, :, 64:65], ones_c[:])

                for wdram, mode in ((wqT3, "q"), (wkT3, "k"),
                                    (wvT3, "v")):
                    w_sb = wpool.tile([P, KT, 512], MMD, tag="w",
                                      name=f"w_{mode}")
                    for k in range(KT):
                        nc.sync.dma_start(out=w_sb[:, k, :],
                                            in_=wdram[:, k, :])
                    for sb in range(SB):
                        sbs = slice(sb * 512, (sb + 1) * 512)
                        xs = xpool.tile([P, KT, 512], MMD, tag="xs",
                                        name="xs")
                        # split so the k-loop can start on the first slice
                        for kq in range(4):
                            nc.sync.dma_start(
                                out=xs[:, 2 * kq:2 * kq + 2, :],
                                in_=xT3[:, 2 * kq:2 * kq + 2, sbs])
                        if mode in ("q", "k"):
                            for hp in range(HP):
                                hps = slice(hp * 128, (hp + 1) * 128)
                                ps = psA.tile([P, 512], FP32, tag="pa",
                                              name="pa")
                                for k in range(KT):
                                    nc.tensor.matmul(
                                        ps[:, :],
                                        w_sb[:, k, hps],
                                        xs[:, k, :],
                                        start=(k == 0), stop=(k == KT - 1),
                                    )
                                rot = rpool.tile([P, 512], FP32, tag="rot",
                                                 name="rot")
                                nc.vector.stream_shuffle(rot[:], ps[:, :],
                                                         mask=SHUF16)
                                t1 = rpool.tile([P, 512], FP32, tag="t1",
                                                name="t1")
                                nc.vector.tensor_mul(t1[:], ps[:, :],
                                                     cos_sb[:, sbs])
                                t2 = rpool.tile([P, 512], FP32, tag="t2",
                                                name="t2")
                                nc.vector.tensor_mul(t2[:], rot[:],
                                                     sin_sb[:, sbs])
                                if mode == "q":
                                    nc.vector.tensor_add(qt[hp][:, sbs],
                                                         t1[:], t2[:])
                                else:
                                    nc.vector.tensor_add(kt[hp][:, sbs],
                                                         t1[:], t2[:])
                        else:
                            for sti in range(4):
                                st = sb * 4 + sti
                                stp = slice(sti * 128, (sti + 1) * 128)
                                psv = psA.tile([P, 512], FP32, tag="pa",
                                               name="psv")
                                for k in range(KT):
                                    nc.tensor.matmul(
                                        psv[:, :],
                                        xs[:, k, stp],
                                        w_sb[:, k, :],
                                        start=(k == 0), stop=(k == KT - 1),
                                    )
                                pv = psv[:, :].rearrange(
                                    "p (h d) -> p h d", h=8)
                                # every head: [V | 1]
                                nc.scalar.copy(v_sb[:, st, :, 0:64],
                                               pv[:, :, :])

            # ---------------- Phase B: attention ----------------
            ot_pool = ctx.enter_context(tc.tile_pool(name="otp", bufs=1))
            oT = [ot_pool.tile([P, S], MMD, tag=f"oT{i}", name=f"oT{i}")
                  for i in range(HP)]
            wopool = ctx.enter_context(tc.tile_pool(name="wo", bufs=1))
            wo_sb = wopool.tile([P, 4, D_MODEL], MMD)
            nc.sync.dma_start(out=wo_sb[:], in_=woT3[:, :, :])

            with ExitStack() as bctx:
                epool = bctx.enter_context(tc.tile_pool(name="expp", bufs=4))
                rdpool = bctx.enter_context(tc.tile_pool(name="rdp", bufs=2))
                dspool = bctx.enter_context(tc.tile_pool(name="dsp", bufs=1))
                bcpool = bctx.enter_context(tc.tile_pool(name="bcp", bufs=2))
                psS = bctx.enter_context(
                    tc.tile_pool(name="psS", bufs=2, space="PSUM"))
                # po tiles are staged out to SBUF right after attn@V, so a
                # single buffer per head suffices
                psO = bctx.enter_context(
                    tc.tile_pool(name="psO", bufs=1, space="PSUM"))
                # output projection accumulator: own pool so the scores
                # ring never waits on its eviction
                psC = bctx.enter_context(
                    tc.tile_pool(name="psC", bufs=1, space="PSUM"))

                def nlo_of(I, J):
                    r = I - 4 * J
                    return 128 * r if r >= 0 else 0

                opool = bctx.enter_context(
                    tc.tile_pool(name="ostage", bufs=1))

                def emit_outproj_st(st):
                    # output projection for one 128-query s-tile; psum
                    # borrowed from the scores pool (same tag)
                    stp = slice(st * 128, (st + 1) * 128)
                    pc = psC.tile([P, 2, 512], FP32, tag="pc", name="pc")
                    for nb in range(2):
                        nbs = slice(nb * 512, (nb + 1) * 512)
                        for t in range(4):
                            nc.tensor.matmul(
                                pc[:, nb, :],
                                oT[t][:, stp],
                                wo_sb[:, t, nbs],
                                start=(t == 0), stop=(t == 3),
                            )
                    ob = opool.tile([P, 2, 512], FP32, tag="ob", name="ob")
                    nc.vector.tensor_copy(ob[:], pc[:, :])
                    nc.gpsimd.dma_start(
                        out=outD[stp, :],
                        in_=ob[:, :, :].rearrange("p a b -> p (a b)"))

                # round-robin over head pairs; the output projection for a
                # J-group is deferred into the NEXT group (one s-tile per
                # hp-block) so the PE never waits on the softmax-denominator
                # chains.  Denominator reciprocals are batched per J-group:
                # the 8 [1,512] rows are staged, DMA-gathered onto 8
                # partitions, and inverted with ONE DVE reciprocal (its cost
                # is free-size-bound, so 8 rows cost the same as 1).
                jseq = (0, 1, 2, 3)
                pending = []

                def pop_outproj():
                    if pending:
                        emit_outproj_st(pending.pop(0))

                for jidx, J in enumerate(jseq):
                    Js = slice(J * 512, (J + 1) * 512)
                    # [V|1] attn@V results staged out of PSUM per (hp,h2):
                    # rows 0:64 head dims, row 64 the softmax denominator
                    stage = dspool.tile([P, 8, 512], FP32, tag="dstage",
                                        name="dstage")

                    def emit_chain(half):
                        # softmax-denominator chain for head pairs
                        # (2*half, 2*half+1): gather the 4 staged rows onto
                        # partitions 0:4 via a DRAM bounce, invert once
                        # (DVE reciprocal cost is free-size-bound), bounce
                        # back out for the per-row partition broadcasts.
                        hs = slice(4 * half, 4 * half + 4)
                        nc.sync.dma_start(out=scrD[J, half, :, :],
                                          in_=stage[64:65, hs, :])
                        dg = rdpool.tile([4, 512], FP32, tag="dg",
                                         name="dg")
                        nc.sync.dma_start(out=dg[0:4, :],
                                          in_=scrD[J, half, :, :])
                        rd = rdpool.tile([4, 512], FP32, tag="rd",
                                         name="rd")
                        nc.vector.reciprocal(rd[0:4, :], dg[0:4, :])
                        nc.sync.dma_start(out=scrR[J, half, :, :],
                                          in_=rd[0:4, :])
                        for hp in (2 * half, 2 * half + 1):
                            for h2 in range(2):
                                idx = hp * 2 + h2
                                bc = bcpool.tile([P, 512], FP32, tag="bc",
                                                 name="bc")
                                eng = nc.gpsimd if h2 == 0 else nc.sync
                                eng.dma_start(
                                    out=bc[0:64, :],
                                    in_=scrR[J, half, idx - 4 * half, :]
                                    .partition_broadcast(64))
                                if h2 == 0:
                                    nc.vector.tensor_mul(
                                        oT[hp][0:64, Js],
                                        stage[0:64, idx, :],
                                        bc[0:64, :])
                                else:
                                    # normalized evict lands at partitions
                                    # 0-63; DMA shifts it into oT's upper
                                    # half
                                    tmp = rdpool.tile([P, 512], MMD,
                                                      tag="tmpb",
                                                      name="tmpb")
                                    nc.vector.tensor_mul(
                                        tmp[0:64, :],
                                        stage[0:64, idx, :],
                                        bc[0:64, :])
                                    nc.sync.dma_start(
                                        out=oT[hp][64:128, Js],
                                        in_=tmp[0:64, :])

                    n_i = 4 * J + 4
                    for hp in range(HP):
                        if hp >= 1:
                            pop_outproj()
                        po = [psO.tile([P, 512], FP32, tag=f"po{h2}",
                                       name=f"po{h2}") for h2 in range(2)]

                        def emit_attnv(I, ex):
                            nlo = nlo_of(I, J)
                            for h2 in range(2):
                                # attn @ [V|1]: rows 0..63 dims, row 64
                                # denom
                                nc.tensor.matmul(
                                    po[h2][0:65, nlo:],
                                    v_sb[:, I, hp * 2 + h2, :],
                                    ex[:, h2, nlo:],
                                    start=(I == 0), stop=(I == n_i - 1),
                                )

                        # attn@V trails the scores by two I-steps so the
                        # PE fills the exp latency with upcoming scores
                        # (the inner loop is otherwise ACT-gated)
                        back = []
                        for I in range(n_i):
                            nlo = nlo_of(I, J)
                            ks = slice(I * 128, (I + 1) * 128)
                            qs = slice(J * 512 + nlo, (J + 1) * 512)
                            diag = I - 4 * J >= 0
                            ps = psS.tile([P, 2, 512], FP32, tag="psS",
                                          name="psS")
                            for h2 in range(2):
                                hr = slice(h2 * 64, (h2 + 1) * 64)
                                nc.tensor.matmul(
                                    ps[:, h2, nlo:],
                                    kt[hp][hr, ks],
                                    qt[hp][hr, qs],
                                    start=True, stop=not diag,
                                )
                                if diag:
                                    # causal corner: accumulate -2^30 into
                                    # masked (q < k) entries, exp -> 0
                                    nc.tensor.matmul(
                                        ps[:, h2, nlo:nlo + 128],
                                        ident_sb[:, :],
                                        maskadd_sb[:, :],
                                        start=False, stop=True,
                                    )
                            ex = epool.tile([P, 2, 512], MMD, tag="ex",
                                            name="ex")
                            nc.scalar.activation(ex[:, :, nlo:],
                                                 ps[:, :, nlo:],
                                                 EXP, scale=0.125)
                            back.append((I, ex))
                            if len(back) > 2:
                                emit_attnv(*back.pop(0))
                        for item in back:
                            emit_attnv(*item)
                        for h2 in range(2):
                            # free the PSUM slot right away: dims + denom
                            # staged to SBUF in one copy
                            nc.scalar.copy(
                                stage[0:65, hp * 2 + h2, :],
                                po[h2][0:65, :])
                        if hp % 2 == 1:
                            emit_chain(hp // 2)
                    pop_outproj()
                    pending.extend(range(4 * J, 4 * J + 4))
                while pending:
                    pop_outproj()

            if taps:
                nc.gpsimd.dma_start(out=tq[:, :], in_=qt[0][:])
                nc.gpsimd.dma_start(out=tk[:, :], in_=kt[0][:])
                nc.gpsimd.dma_start(out=tv[:, :, :, :], in_=v_sb[:])
                nc.gpsimd.dma_start(out=to[:, :], in_=oT[0][:])

    nc.compile()
    return nc


def get_module(mm="float32r"):
    if mm not in _CACHE:
        _CACHE[mm] = _build_module(mm)
    return _CACHE[mm]


def _head_perm():
    """Within-head dim permutation: 16-pair blocks [x1 x2 x1 x2]."""
    p = []
    for blk in range(2):
        base = blk * 32
        p += [2 * (base // 2 + i) for i in range(16)]       # x1 of pairs
        p += [2 * (base // 2 + i) + 1 for i in range(16)]   # x2 of pairs
    return np.array(p)


def prep_core_inputs(inputs, mm="float32r"):
    import ml_dtypes
    mdt = np.float32 if mm != "bfloat16" else ml_dtypes.bfloat16
    x = np.asarray(inputs["x"], dtype=np.float32)
    tp = np.asarray(inputs["token_positions"])
    wq = np.asarray(inputs["wq"], dtype=np.float32)
    wk = np.asarray(inputs["wk"], dtype=np.float32)
    wv = np.asarray(inputs["wv"], dtype=np.float32)
    wo = np.asarray(inputs["wo"], dtype=np.float32)

    import ml_dtypes
    perm = _head_perm()
    qi = np.arange(128)[None, :]
    ki = np.arange(128)[:, None]
    mask_add = np.where(qi < ki, -np.float32(2.0 ** 30),
                        np.float32(0.0)).astype(ml_dtypes.bfloat16)
    ident = np.eye(128, dtype=ml_dtypes.bfloat16)

    freqs = 1.0 / THETA ** (np.arange(0, D_K, 2, dtype=np.float32) / D_K)

    in_maps = []
    for c in range(N_CORES):
        b, g = divmod(c, 2)
        rows = slice(g * 512, (g + 1) * 512)
        wq_g = wq[rows].reshape(H_CORE, D_K, D_MODEL)[:, perm, :]
        wk_g = wk[rows].reshape(H_CORE, D_K, D_MODEL)[:, perm, :]

        pos = tp[b].astype(np.float32)
        ang = freqs[:, None] * pos[None, :]          # [32, S]
        cos32, sin32 = np.cos(ang), np.sin(ang)
        # permuted row l: l%32 < 16 -> x1 of pair (l%32 + 16*(l//32)),
        #                 else x2 of the same pair; x1 rows get -sin.
        cos64 = np.concatenate([cos32[0:16], cos32[0:16],
                                cos32[16:32], cos32[16:32]], axis=0)
        sin64 = np.concatenate([-sin32[0:16], sin32[0:16],
                                -sin32[16:32], sin32[16:32]], axis=0)
        cosT = np.tile(cos64, (2, 1))
        sinT = np.tile(sin64, (2, 1))

        in_maps.append({
            "xT": np.ascontiguousarray(x[b].T).astype(mdt),
            "wqT": np.ascontiguousarray(wq_g.reshape(512, D_MODEL).T).astype(mdt),
            "wkT": np.ascontiguousarray(wk_g.reshape(512, D_MODEL).T).astype(mdt),
            "wvT": np.ascontiguousarray(wv[rows].T).astype(mdt),
            "woT": np.ascontiguousarray(wo[:, rows].T).astype(mdt),
            "cosT": np.ascontiguousarray(cosT),
            "sinT": np.ascontiguousarray(sinT),
            "maskA": mask_add,
            "identT": ident,
        })
    return in_maps


DEFAULT_MM = "float32r"


def kernel(**inputs):
    from concourse.bass_utils import run_bass_kernel_spmd

    import os
    mm = os.environ.get("KMM", DEFAULT_MM)
    nc = get_module(mm)
    in_maps = prep_core_inputs(inputs, mm)
    res = run_bass_kernel_spmd(nc, in_maps, core_ids=list(range(N_CORES)))
    out = np.empty((B_FULL, S, D_MODEL), np.float32)
    for b in range(B_FULL):
        out[b] = res.results[2 * b]["out"] + res.results[2 * b + 1]["out"]
    return out
